# revision 1
# baseline (speedup 1.0000x reference)
"""Distributed Trainium2 kernel for nn_AdjMatmulEncoder.

Strategy (per the sharding hint): pure data parallelism over the bsz axis.
bsz=8 maps 1:1 onto the 8 NeuronCores; every device holds the full
parameter set (replicated, they are tiny vs. the (n,n,bsz,h) activations)
and its own bsz=1 shard of rel_type. All attention / einsum intermediates
carry bsz, so no cross-device communication is needed inside the layers;
the full output is reassembled on the host by concatenating the 8 shards.
"""

import numpy as np
import jax
import jax.numpy as jnp

N, BSZ, HID, HEADS, DH = 96, 8, 512, 8, 64
VOCAB, REL_DIM, EMBED, MAX_M, PAD_IDX = 100, 128, 512, 2, 0
LN_EPS = 1e-5
_SCALE = DH ** -0.5

_ARG_ORDER = (
    "src_tokens", "rel_type", "rel_embed", "rel_proj_w", "rel_proj_b",
    "wq", "bq", "wk", "bk", "path_w", "path_b", "ln_g", "ln_b",
    "out_w", "out_b",
)

_pmapped = None


def _shard_fn(src_tokens, rel_type, rel_embed, rel_proj_w, rel_proj_b,
              wq, bq, wk, bk, path_w, path_b, ln_g, ln_b, out_w, out_b):
    # rel_type arrives as this device's (N, N) shard (bsz axis consumed by pmap).
    rel_type = rel_type[:, :, None]                       # (N, N, 1)
    emb = rel_embed[src_tokens[0]]                        # (L, rel_dim)
    rel_base = emb @ rel_proj_w.T + rel_proj_b            # (L, hid)
    relation = rel_base[rel_type]                         # (N, N, 1, hid)
    state = relation
    attn_mask = rel_type == PAD_IDX
    diag = jnp.eye(N, dtype=bool)[:, :, None]
    attn_mask = jnp.where(diag, False, attn_mask)
    km = jnp.transpose(attn_mask, (0, 2, 1))[:, :, None, None, :]  # (i,b,1,1,s)

    for _ in range(min(MAX_M, N)):
        q = (jnp.einsum("itbh,oh->itbo", state, wq) + bq) * _SCALE
        k = jnp.einsum("isbh,oh->isbo", state, wk) + bk
        q = q.reshape(N, N, 1, HEADS, DH)
        k = k.reshape(N, N, 1, HEADS, DH)
        scores = jnp.einsum("itbed,isbed->ibets", q, k)
        scores = jnp.where(km, -jnp.inf, scores)
        attn = jax.nn.softmax(scores, axis=-1).mean(axis=2)   # (i, b, t, s)
        x = jnp.einsum("ibts,isbh->itbh", attn, state)
        rel_mix = jnp.einsum("ibts,stbh->itbh", attn, relation)
        out = jnp.einsum(
            "itbc,oc->itbo", jnp.concatenate([x, rel_mix], axis=-1), path_w
        ) + path_b
        out = jax.nn.relu(out)
        res = state + out
        mu = res.mean(axis=-1, keepdims=True)
        var = res.var(axis=-1, keepdims=True)
        state = (res - mu) * jax.lax.rsqrt(var + LN_EPS) * ln_g + ln_b

    out = jnp.einsum("itbh,oh->itbo", state, out_w) + out_b   # (N, N, 1, EMBED)
    return out[:, :, 0, :]                                    # (N, N, EMBED)


def _build():
    devices = jax.devices()[:BSZ]
    in_axes = (None, 2) + (None,) * 13
    return jax.pmap(_shard_fn, in_axes=in_axes, out_axes=2, devices=devices)


def kernel(**inputs: np.ndarray) -> np.ndarray:
    global _pmapped
    if _pmapped is None:
        _pmapped = _build()
    args = [np.asarray(inputs[k]) for k in _ARG_ORDER]
    out = _pmapped(*args)                                     # (N, N, BSZ, EMBED)
    return np.asarray(out, dtype=np.float32)

